# revision 10
# baseline (speedup 1.0000x reference)
"""Trainium2 Bass kernel for nn_Attention_LoRA (Swin attention w/ LoRA + rel-pos bias).

v2 design (vs v1: fp32r + 256-pad + per-batch head units):
  - bf16 matmul inputs, fp16 softmax intermediates (DVE 2x mode), f32 psums.
  - No token padding: the two batches of a pair are PACKED along the free
    axis (197+197=394), so every matmul/elementwise op streams only real
    tokens. fp32r's free>=256 restriction is gone with 16-bit inputs.
  - One head unit covers BOTH batches of the pair: half the units, half the
    recip/broadcast/normalize ops of v1.
  - v-bias (the softmax-denominator ones columns) applied during the DVE
    PSUM->SBUF evacuation instead of an extra PE matmul; proj bias applied
    by a rank-1 fp16 matmul folded into the proj accumulation.
  Per batch: qkT[c,t] = Wqk @ xT; v[t,c'] = x @ WvT (65-wide head blocks,
  ones col -> PV row 64 = softmax denom l); ST[k,q] per head; PT = exp(ST)
  * exp(rpb); OT = v.T @ PT; ao = OT[0:64]/l; y = ao.T @ projW + pb.
  Measured on trn2 (8 cores, axon, paired For_i-delta R2=8192): ~364 us per
  core for its 8-batch share (v1 fp32r baseline: 535-596 us same method);
  scaled absmax error vs fp32 reference ~4.4e-3.
"""
import os
from contextlib import ExitStack

import numpy as np

import concourse.bacc as bacc
import concourse.bass as bass
import concourse.mybir as mybir
import concourse.tile as tile
from concourse import bass_utils

B, NT, C, H, WS, RANK = 64, 197, 768, 12, 14, 24
HD = C // H
SCALE = HD ** -0.5
NCORES = 8
BPC = B // NCORES          # batches per core
NN = NT                    # real token count (no padding)
F2 = 2 * NN                # 394: two batches packed along the free axis
KC = C // 128              # 6 contraction chunks
NCO = (2 * C) // 128       # 12 q+k output chunks
VW = H * (HD + 1)          # 780: v head-blocks of 65 (64 v cols + ones col)
VH = VW // 2               # 390
PH = C // 2                # 384
KT_SZ = [128, NT - 128]    # keys chunks: 128, 69
TT_SZ = [128, NT - 128]    # token chunks: 128, 69

PERF_REPS = int(os.environ.get("PERF_REPS", "0"))
NORM_MODE = os.environ.get("NORM_MODE", "gps")  # gps | dram | dvecopy | off
SKIP_MODE = os.environ.get("SKIP_MODE", "full")  # full | dense (timing probe)

_prog_cache = {}


def _build_program():
    key = (PERF_REPS, NORM_MODE, SKIP_MODE)
    if key in _prog_cache:
        return _prog_cache[key]

    f32 = mybir.dt.float32
    f16 = mybir.dt.float16
    mdt = mybir.dt.bfloat16
    Exp = mybir.ActivationFunctionType.Exp

    nc = bacc.Bacc("TRN2", target_bir_lowering=False, debug=False,
                   num_devices=NCORES)
    xt_d = nc.dram_tensor("xt", [BPC // 2, C, F2], mdt,
                          kind="ExternalInput").ap()
    wqk_d = nc.dram_tensor("wqk", [C, 2 * C], mdt, kind="ExternalInput").ap()
    bqk_d = nc.dram_tensor("bqk", [128, NCO], f32, kind="ExternalInput").ap()
    wv_d = nc.dram_tensor("wv", [C, VW], mdt, kind="ExternalInput").ap()
    vb_d = nc.dram_tensor("vb", [1, VW], f32, kind="ExternalInput").ap()
    pw_d = nc.dram_tensor("pw", [C, C], mdt, kind="ExternalInput").ap()
    pb_d = nc.dram_tensor("pb", [1, C], f16, kind="ExternalInput").ap()
    ones_d = nc.dram_tensor("ones", [1, 128], f16, kind="ExternalInput").ap()
    erpb_d = nc.dram_tensor("erpb", [H, 128, 2 * F2], f16,
                            kind="ExternalInput").ap()
    rs_d = nc.dram_tensor("rs", [16, 1, F2], f32, kind="Internal").ap()
    y_d = nc.dram_tensor("y", [BPC, NT, C], mdt, kind="ExternalOutput").ap()

    with tile.TileContext(nc) as tc, ExitStack() as ctx, \
            nc.allow_low_precision("bf16/fp16 matmul + softmax by design"):
        consts = ctx.enter_context(tc.tile_pool(name="consts", bufs=1))
        xp = ctx.enter_context(tc.tile_pool(name="xp", bufs=1))
        qkp = ctx.enter_context(tc.tile_pool(name="qkp", bufs=1))
        vp = ctx.enter_context(tc.tile_pool(name="vp", bufs=1))
        ep = ctx.enter_context(tc.tile_pool(name="ep", bufs=2))
        ptp = ctx.enter_context(tc.tile_pool(name="ptp", bufs=2))
        lrp = ctx.enter_context(tc.tile_pool(name="lrp", bufs=3))
        rfp = ctx.enter_context(tc.tile_pool(name="rfp", bufs=3))
        aop = ctx.enter_context(tc.tile_pool(name="aop", bufs=1))
        yp = ctx.enter_context(tc.tile_pool(name="yp", bufs=2))
        psA = ctx.enter_context(tc.tile_pool(name="psA", bufs=2, space="PSUM"))
        psS = ctx.enter_context(tc.tile_pool(
            name="psS", bufs=(1 if NORM_MODE == "pemm" else 2), space="PSUM"))
        psO = ctx.enter_context(tc.tile_pool(name="psO", bufs=2, space="PSUM"))
        psR = (ctx.enter_context(tc.tile_pool(name="psR", bufs=2, space="PSUM"))
               if NORM_MODE == "pemm" else None)

        # ---- constants ----
        wqk_sb = []
        wv_sb = []
        pw_sb = []
        for kc in range(KC):
            t = consts.tile([128, 2 * C], mdt, tag=f"wqk{kc}", name=f"wqk{kc}")
            nc.sync.dma_start(out=t, in_=wqk_d[kc * 128:(kc + 1) * 128, :])
            wqk_sb.append(t)
        for kc in range(KC):
            t = consts.tile([128, VW], mdt, tag=f"wv{kc}", name=f"wv{kc}")
            nc.sync.dma_start(out=t, in_=wv_d[kc * 128:(kc + 1) * 128, :])
            wv_sb.append(t)
        for kc in range(KC):
            t = consts.tile([128, C], mdt, tag=f"pw{kc}", name=f"pw{kc}")
            nc.sync.dma_start(out=t, in_=pw_d[kc * 128:(kc + 1) * 128, :])
            pw_sb.append(t)
        vb_bcast = consts.tile([128, VW], f32, tag="vbb", name="vbb")
        nc.sync.dma_start(out=vb_bcast, in_=bass.AP(
            tensor=vb_d.tensor, offset=vb_d.offset,
            ap=[[0, 128]] + list(vb_d.ap[1:])))
        pb_sb = consts.tile([1, C], f16, tag="pb", name="pb")
        nc.sync.dma_start(out=pb_sb, in_=pb_d)
        ones_sb = consts.tile([1, 128], f16, tag="ones", name="ones")
        nc.sync.dma_start(out=ones_sb, in_=ones_d)
        bqk_sb = consts.tile([128, NCO], f32, tag="bqk", name="bqk")
        nc.sync.dma_start(out=bqk_sb, in_=bqk_d)
        erpb_sb = {}
        for h in range(H):
            t = consts.tile([128, 2 * F2], f16, tag=f"erpb{h}",
                            name=f"erpb{h}")
            nc.sync.dma_start(out=t, in_=erpb_d[h, :, :])
            erpb_sb[h] = t

        PAIRS = BPC // 2
        co_order = [c for pr in zip(range(KC), range(KC, NCO)) for c in pr]

        def build_A(p):
            """Emit pair p's xt DMAs now; return (qk_sb, xt_sb, thunks)."""
            par = p % 2
            xt_sb = []
            for kc in range(KC):
                t = xp.tile([128, F2], mdt, tag=f"x{par}_{kc}",
                            name=f"x{par}_{kc}")
                nc.sync.dma_start(out=t, in_=xt_d[p, kc * 128:(kc + 1) * 128, :])
                xt_sb.append(t)
            qk_sb = [None] * NCO

            def qk_thunk(co):
                ps = psA.tile([128, F2], f32, tag="mm", name="mm")
                for kc in range(KC):
                    nc.tensor.matmul(
                        ps, wqk_sb[kc][:, co * 128:(co + 1) * 128], xt_sb[kc],
                        start=(kc == 0), stop=(kc == KC - 1))
                qk = qkp.tile([128, F2], mdt, tag=f"qk{par}_{co}",
                              name=f"qk{par}_{co}")
                nc.scalar.add(qk, ps, add=bqk_sb[:, co:co + 1])
                qk_sb[co] = qk

            thunks = [lambda co=co: qk_thunk(co) for co in co_order]
            return qk_sb, xt_sb, thunks

        def emit_V(xt_sb, par):
            """v[b][tt] = [tokens<=128, 780] bf16 (ones cols via vb add)."""
            v_sbs = [[None, None], [None, None]]
            for b in range(2):
                for tt in range(2):
                    TL = TT_SZ[tt]
                    xo = b * NN + tt * 128
                    v = vp.tile([128, VW], mdt, tag=f"v{par}_{b}_{tt}",
                                name=f"v{par}_{b}_{tt}")
                    for half in range(2):
                        ps = psA.tile([128, VH], f32, tag="mm", name="mm")
                        for kc in range(KC):
                            nc.tensor.matmul(
                                ps[0:TL], xt_sb[kc][:, xo:xo + TL],
                                wv_sb[kc][:, half * VH:(half + 1) * VH],
                                start=(kc == 0), stop=(kc == KC - 1))
                        nc.vector.tensor_add(
                            v[0:TL, half * VH:(half + 1) * VH], ps[0:TL],
                            vb_bcast[0:TL, half * VH:(half + 1) * VH])
                    v_sbs[b][tt] = v
            return v_sbs

        rs_slot = [0]

        def head_unit(h, qk_sb, v_sbs, ao_sb):
            qq = qk_sb[h // 2]
            kk = qk_sb[KC + h // 2]
            po = (h % 2) * 64
            st = []
            for kt in range(2):
                KT = KT_SZ[kt]
                s = psS.tile([128, F2], f32, tag=f"st{kt}", name=f"st{kt}")
                for b in range(2):
                    nc.tensor.matmul(
                        s[0:KT, b * NN:(b + 1) * NN],
                        kk[po:po + 64, b * NN + kt * 128:b * NN + kt * 128 + KT],
                        qq[po:po + 64, b * NN:(b + 1) * NN],
                        start=(b == 0), stop=(b == 1))
                st.append(s)
            e = ep.tile([128, 2 * F2], f16, tag="e", name="e")
            for kt in range(2):
                nc.scalar.activation(e[:, kt * F2:(kt + 1) * F2], st[kt], Exp)
            ptt = ptp.tile([128, 2 * F2], f16, tag="pt", name="pt")
            nc.vector.tensor_mul(ptt, e, erpb_sb[h])
            pt = [ptt[:, 0:F2], ptt[:, F2:2 * F2]]
            ot = psO.tile([65, F2], f32, tag="ot", name="ot")
            first = True
            for b in range(2):
                for kt in range(2):
                    KT = KT_SZ[kt]
                    nc.tensor.matmul(
                        ot[:, b * NN:(b + 1) * NN],
                        v_sbs[b][kt][0:KT, h * 65:(h + 1) * 65],
                        pt[kt][0:KT, b * NN:(b + 1) * NN],
                        start=first, stop=(b == 1 and kt == 1))
                    first = False
            if NORM_MODE == "off":
                nc.scalar.copy(ao_sb[h // 2][po:po + 64, :], ot[0:64, :])
                return
            if NORM_MODE == "pemm":
                r16 = lrp.tile([1, F2], f16, tag="r16", name="r16")
                nc.vector.reciprocal(r16, ot[64:65, :])
                r_ps = psR.tile([64, F2], f32, tag="rps", name="rps")
                nc.tensor.matmul(r_ps, ones_sb[:, 0:64], r16,
                                 start=True, stop=True)
                nc.vector.tensor_mul(ao_sb[h // 2][po:po + 64, :],
                                     ot[0:64, :], r_ps)
                return
            r_sb = lrp.tile([1, F2], f32, tag="r", name="r")
            nc.vector.reciprocal(r_sb, ot[64:65, :])
            if NORM_MODE == "bcastap":
                rs = r_sb[0:1, :]
                nc.vector.tensor_mul(
                    ao_sb[h // 2][po:po + 64, :], ot[0:64, :],
                    bass.AP(tensor=rs.tensor, offset=rs.offset,
                            ap=[[0, 64]] + list(rs.ap[1:])))
                return
            r_full = rfp.tile([64, F2], f32, tag="rf", name="rf")
            if NORM_MODE == "dvecopy":
                nc.vector.tensor_copy(r_full, vb_bcast[0:64, 0:F2])
            elif NORM_MODE == "dram":
                slot = rs_slot[0] % 16
                rs_slot[0] += 1
                nc.sync.dma_start(out=rs_d[slot], in_=r_sb)
                rd = rs_d[slot][0:1, :]
                nc.sync.dma_start(out=r_full, in_=bass.AP(
                    tensor=rd.tensor, offset=rd.offset,
                    ap=[[0, 64]] + list(rd.ap[1:])))
            else:
                nc.gpsimd.partition_broadcast(r_full, r_sb)
            nc.vector.tensor_mul(ao_sb[h // 2][po:po + 64, :],
                                 ot[0:64, :], r_full)

        def emit_proj(p, ao_sb):
            for b in range(2):
                for tt in range(2):
                    t0 = tt * 128
                    tl = TT_SZ[tt]
                    y_sb = yp.tile([128, C], mdt, tag="y", name="y")
                    for half in range(2):
                        ps = psA.tile([128, PH], f32, tag="mm", name="mm")
                        nc.tensor.matmul(
                            ps[0:tl], ones_sb[:, 0:tl],
                            pb_sb[:, half * PH:(half + 1) * PH],
                            start=True, stop=False)
                        for dc in range(KC):
                            nc.tensor.matmul(
                                ps[0:tl], ao_sb[dc][:, b * NN + t0:b * NN + t0 + tl],
                                pw_sb[dc][:, half * PH:(half + 1) * PH],
                                start=False, stop=(dc == KC - 1))
                        nc.scalar.copy(
                            y_sb[0:tl, half * PH:(half + 1) * PH], ps[0:tl])
                    nc.sync.dma_start(
                        out=y_d[2 * p + b, t0:t0 + tl, :], in_=y_sb[0:tl])

        def whole_pass():
            qk_cur, xt_cur, thunks = build_A(0)
            for t in thunks:
                t()
            for p in range(PAIRS):
                par = p % 2
                v_cur = emit_V(xt_cur, par)
                if p + 1 < PAIRS:
                    qk_nxt, xt_nxt, a_thunks = build_A(p + 1)
                else:
                    qk_nxt = xt_nxt = None
                    a_thunks = []
                ao_sb = [aop.tile([128, F2], mdt, tag=f"ao{par}_{dc}",
                                  name=f"ao{par}_{dc}") for dc in range(KC)]
                if SKIP_MODE == "dense":
                    # timing probe: skip attention, fill ao with junk
                    for dc in range(KC):
                        nc.vector.tensor_copy(ao_sb[dc], vb_bcast[:, 0:F2])
                    for t in a_thunks:
                        t()
                else:
                    emitted = 0
                    for h in range(H):
                        head_unit(h, qk_cur, v_cur, ao_sb)
                        want = (h + 1) * len(a_thunks) // H
                        while emitted < want:
                            a_thunks[emitted]()
                            emitted += 1
                emit_proj(p, ao_sb)
                qk_cur, xt_cur = qk_nxt, xt_nxt

        if PERF_REPS > 0:
            with tc.For_i(0, PERF_REPS, 1):
                whole_pass()
        else:
            for _ in range(int(os.environ.get("PERF_UNROLL", "1"))):
                whole_pass()

    nc.compile()
    _prog_cache[key] = nc
    return nc


def _host_prep(x, qkv_w, q_bias, v_bias, q_lora_a, q_lora_b, k_lora_a,
               k_lora_b, v_lora_a, v_lora_b, rel_pos_table, proj_w, proj_b,
               rel_pos_index):
    import ml_dtypes
    bf16 = ml_dtypes.bfloat16
    f = np.float32
    x = np.asarray(x, f)
    q_bias = np.asarray(q_bias, f)
    proj_w = np.asarray(proj_w, f)
    rel_pos_table = np.asarray(rel_pos_table, f)
    rel_pos_index = np.asarray(rel_pos_index)

    # fold LoRA (x @ A.T @ B.T == x @ (B@A).T) and attention scale into weights
    lora = np.vstack([
        np.asarray(q_lora_b, np.float64) @ np.asarray(q_lora_a, np.float64),
        np.asarray(k_lora_b, np.float64) @ np.asarray(k_lora_a, np.float64),
        np.asarray(v_lora_b, np.float64) @ np.asarray(v_lora_a, np.float64),
    ])
    W = (np.asarray(qkv_w, np.float64) + lora)
    W[0:C] *= SCALE
    W = W.astype(f)

    wqk = np.ascontiguousarray(W[0:2 * C].T)                     # [768, 1536]
    bqk = np.ascontiguousarray(
        np.concatenate([q_bias * SCALE, np.zeros(C, f)]).reshape(NCO, 128).T)

    WvT = W[2 * C:3 * C].T                                       # [768, 768]
    wv = np.zeros((C, VW), f)
    vb = np.zeros((1, VW), f)
    for h in range(H):
        wv[:, h * 65:h * 65 + 64] = WvT[:, h * 64:(h + 1) * 64]
        vb[0, h * 65 + 64] = 1.0
    pw = np.ascontiguousarray(proj_w.T)
    # softmax weights sum to 1 -> v_bias adds a constant to attn_out;
    # fold it into the projection bias: pb = proj_b + proj_w @ v_bias
    pb = (np.asarray(proj_b, f) + proj_w @ np.asarray(v_bias, f)).reshape(1, C)

    # exp(rpb): [h, kt, k_in_chunk, q] duplicated for the two packed batches
    rpb = rel_pos_table[rel_pos_index.reshape(-1).astype(np.int64)]
    rpb = rpb.reshape(NT, NT, H)                                  # [q, k, h]
    erpb_t = np.exp(rpb).transpose(2, 1, 0).astype(f)             # [h, k, q]
    erpb = np.ones((H, 2, 128, NN), f)
    erpb[:, 0, 0:128, :] = erpb_t[:, 0:128, :]
    erpb[:, 1, 0:NT - 128, :] = erpb_t[:, 128:NT, :]
    erpb = np.concatenate([erpb, erpb], axis=3)                   # [h,kt,128,394]
    erpb = np.ascontiguousarray(
        np.concatenate([erpb[:, 0], erpb[:, 1]], axis=2))         # [h,128,788]

    # pack batch pairs side by side along tokens: [B//2, C, 394]
    xt = np.ascontiguousarray(
        x.reshape(B // 2, 2, NN, C).transpose(0, 3, 1, 2).reshape(B // 2, C, F2))

    return {
        "xt": xt.astype(bf16),
        "wqk": wqk.astype(bf16),
        "bqk": bqk,
        "wv": wv.astype(bf16),
        "vb": vb,
        "pw": pw.astype(bf16),
        "pb": pb.astype(np.float16),
        "ones": np.ones((1, 128), np.float16),
        "erpb": erpb.astype(np.float16),
    }


def kernel(**inputs):
    arrs = _host_prep(**inputs)
    nc = _build_program()
    in_maps = []
    ppc = BPC // 2
    for ci in range(NCORES):
        m = dict(arrs)
        m["xt"] = np.ascontiguousarray(arrs["xt"][ci * ppc:(ci + 1) * ppc])
        in_maps.append(m)
    last_exc = None
    for attempt in range(3):
        try:
            res = bass_utils.run_bass_kernel_spmd(
                nc, in_maps, core_ids=list(range(NCORES)))
            break
        except Exception as e:  # transient NRT device flakes recover on retry
            last_exc = e
            import time
            time.sleep(5.0 * (attempt + 1))
    else:
        raise last_exc
    out = np.concatenate([r["y"] for r in res.results], axis=0)
    return out.astype(np.float32)


# revision 11
# speedup vs baseline: 1.2287x; 1.2287x over previous
"""Trainium2 Bass kernel for nn_Attention_LoRA (Swin attention w/ LoRA + rel-pos bias).

v2 design (vs v1: fp32r + 256-pad + per-batch head units):
  - bf16 matmul inputs, fp16 softmax intermediates (DVE 2x mode), f32 psums.
  - No token padding: the two batches of a pair are PACKED along the free
    axis (197+197=394), so every matmul/elementwise op streams only real
    tokens. fp32r's free>=256 restriction is gone with 16-bit inputs.
  - One head unit covers BOTH batches of the pair: half the units, half the
    recip/broadcast/normalize ops of v1.
  - v-bias (the softmax-denominator ones columns) applied during the DVE
    PSUM->SBUF evacuation instead of an extra PE matmul; proj bias applied
    by a rank-1 fp16 matmul folded into the proj accumulation.
  Per batch: qkT[c,t] = Wqk @ xT; v[t,c'] = x @ WvT (65-wide head blocks,
  ones col -> PV row 64 = softmax denom l); ST[k,q] per head; PT = exp(ST)
  * exp(rpb); OT = v.T @ PT; ao = OT[0:64]/l; y = ao.T @ projW + pb.
  Measured on trn2 (8 cores, axon, paired For_i-delta R2=8192): ~364 us per
  core for its 8-batch share (v1 fp32r baseline: 535-596 us same method);
  scaled absmax error vs fp32 reference ~4.4e-3.
"""
import os
from contextlib import ExitStack

import numpy as np

import concourse.bacc as bacc
import concourse.bass as bass
import concourse.mybir as mybir
import concourse.tile as tile
from concourse import bass_utils

B, NT, C, H, WS, RANK = 64, 197, 768, 12, 14, 24
HD = C // H
SCALE = HD ** -0.5
NCORES = 8
BPC = B // NCORES          # batches per core
NN = NT                    # real token count (no padding)
F2 = 2 * NN                # 394: two batches packed along the free axis
KC = C // 128              # 6 contraction chunks
NCO = (2 * C) // 128       # 12 q+k output chunks
VW = H * (HD + 1)          # 780: v head-blocks of 65 (64 v cols + ones col)
VH = VW // 2               # 390
PH = C // 2                # 384
KT_SZ = [128, NT - 128]    # keys chunks: 128, 69
TT_SZ = [128, NT - 128]    # token chunks: 128, 69

PERF_REPS = int(os.environ.get("PERF_REPS", "0"))
NORM_MODE = os.environ.get("NORM_MODE", "gps")  # gps | dram | dvecopy | off
SKIP_MODE = os.environ.get("SKIP_MODE", "full")  # full | dense (timing probe)

_prog_cache = {}


def _build_program():
    key = (PERF_REPS, NORM_MODE, SKIP_MODE)
    if key in _prog_cache:
        return _prog_cache[key]

    f32 = mybir.dt.float32
    f16 = mybir.dt.float16
    mdt = mybir.dt.bfloat16
    Exp = mybir.ActivationFunctionType.Exp

    nc = bacc.Bacc("TRN2", target_bir_lowering=False, debug=False,
                   num_devices=NCORES)
    xt_d = nc.dram_tensor("xt", [BPC // 2, C, F2], mdt,
                          kind="ExternalInput").ap()
    wqk_d = nc.dram_tensor("wqk", [C, 2 * C], mdt, kind="ExternalInput").ap()
    bqk_d = nc.dram_tensor("bqk", [128, NCO], f32, kind="ExternalInput").ap()
    wv_d = nc.dram_tensor("wv", [C, VW], mdt, kind="ExternalInput").ap()
    vb_d = nc.dram_tensor("vb", [1, VW], f32, kind="ExternalInput").ap()
    pw_d = nc.dram_tensor("pw", [C, C], mdt, kind="ExternalInput").ap()
    pb_d = nc.dram_tensor("pb", [1, C], f16, kind="ExternalInput").ap()
    ones_d = nc.dram_tensor("ones", [1, 128], f16, kind="ExternalInput").ap()
    erpb_d = nc.dram_tensor("erpb", [H, 128, 2 * F2], f16,
                            kind="ExternalInput").ap()
    rs_d = nc.dram_tensor("rs", [16, 1, F2], f32, kind="Internal").ap()
    y_d = nc.dram_tensor("y", [BPC, NT, C], mdt, kind="ExternalOutput").ap()

    with tile.TileContext(nc) as tc, ExitStack() as ctx, \
            nc.allow_low_precision("bf16/fp16 matmul + softmax by design"):
        consts = ctx.enter_context(tc.tile_pool(name="consts", bufs=1))
        xp = ctx.enter_context(tc.tile_pool(name="xp", bufs=1))
        qkp = ctx.enter_context(tc.tile_pool(name="qkp", bufs=1))
        vp = ctx.enter_context(tc.tile_pool(name="vp", bufs=1))
        ep = ctx.enter_context(tc.tile_pool(name="ep", bufs=2))
        ptp = ctx.enter_context(tc.tile_pool(name="ptp", bufs=2))
        lrp = ctx.enter_context(tc.tile_pool(name="lrp", bufs=3))
        rfp = ctx.enter_context(tc.tile_pool(name="rfp", bufs=3))
        aop = ctx.enter_context(tc.tile_pool(name="aop", bufs=1))
        yp = ctx.enter_context(tc.tile_pool(name="yp", bufs=2))
        psA = ctx.enter_context(tc.tile_pool(name="psA", bufs=2, space="PSUM"))
        psS = ctx.enter_context(tc.tile_pool(
            name="psS", bufs=(1 if NORM_MODE == "pemm" else 2), space="PSUM"))
        psO = ctx.enter_context(tc.tile_pool(name="psO", bufs=2, space="PSUM"))
        psR = (ctx.enter_context(tc.tile_pool(name="psR", bufs=2, space="PSUM"))
               if NORM_MODE == "pemm" else None)

        # ---- constants ----
        wqk_sb = []
        wv_sb = []
        pw_sb = []
        for kc in range(KC):
            t = consts.tile([128, 2 * C], mdt, tag=f"wqk{kc}", name=f"wqk{kc}")
            nc.sync.dma_start(out=t, in_=wqk_d[kc * 128:(kc + 1) * 128, :])
            wqk_sb.append(t)
        for kc in range(KC):
            t = consts.tile([128, VW], mdt, tag=f"wv{kc}", name=f"wv{kc}")
            nc.sync.dma_start(out=t, in_=wv_d[kc * 128:(kc + 1) * 128, :])
            wv_sb.append(t)
        for kc in range(KC):
            t = consts.tile([128, C], mdt, tag=f"pw{kc}", name=f"pw{kc}")
            nc.sync.dma_start(out=t, in_=pw_d[kc * 128:(kc + 1) * 128, :])
            pw_sb.append(t)
        vb_bcast = consts.tile([128, VW], f32, tag="vbb", name="vbb")
        nc.sync.dma_start(out=vb_bcast, in_=bass.AP(
            tensor=vb_d.tensor, offset=vb_d.offset,
            ap=[[0, 128]] + list(vb_d.ap[1:])))
        pb_sb = consts.tile([1, C], f16, tag="pb", name="pb")
        nc.sync.dma_start(out=pb_sb, in_=pb_d)
        ones_sb = consts.tile([1, 128], f16, tag="ones", name="ones")
        nc.sync.dma_start(out=ones_sb, in_=ones_d)
        bqk_sb = consts.tile([128, NCO], f32, tag="bqk", name="bqk")
        nc.sync.dma_start(out=bqk_sb, in_=bqk_d)
        erpb_sb = {}
        for h in range(H):
            t = consts.tile([128, 2 * F2], f16, tag=f"erpb{h}",
                            name=f"erpb{h}")
            nc.sync.dma_start(out=t, in_=erpb_d[h, :, :])
            erpb_sb[h] = t

        PAIRS = BPC // 2
        co_order = [c for pr in zip(range(KC), range(KC, NCO)) for c in pr]

        def build_A(p):
            """Emit pair p's xt DMAs now; return (qk_sb, xt_sb, thunks)."""
            par = p % 2
            xt_sb = []
            for kc in range(KC):
                t = xp.tile([128, F2], mdt, tag=f"x{par}_{kc}",
                            name=f"x{par}_{kc}")
                nc.sync.dma_start(out=t, in_=xt_d[p, kc * 128:(kc + 1) * 128, :])
                xt_sb.append(t)
            qk_sb = [None] * NCO

            def qk_thunk(co):
                ps = psA.tile([128, F2], f32, tag="mm", name="mm")
                for kc in range(KC):
                    nc.tensor.matmul(
                        ps, wqk_sb[kc][:, co * 128:(co + 1) * 128], xt_sb[kc],
                        start=(kc == 0), stop=(kc == KC - 1))
                qk = qkp.tile([128, F2], mdt, tag=f"qk{par}_{co}",
                              name=f"qk{par}_{co}")
                nc.scalar.add(qk, ps, add=bqk_sb[:, co:co + 1])
                qk_sb[co] = qk

            thunks = [lambda co=co: qk_thunk(co) for co in co_order]
            return qk_sb, xt_sb, thunks

        def emit_V_one(xt_sb, par, b, tt, v_sbs):
            """v[b][tt] = [tokens<=128, 780] bf16 (ones cols via vb add)."""
            TL = TT_SZ[tt]
            xo = b * NN + tt * 128
            v = vp.tile([128, VW], mdt, tag=f"v{par}_{b}_{tt}",
                        name=f"v{par}_{b}_{tt}")
            for half in range(2):
                ps = psA.tile([128, VH], f32, tag="mm", name="mm")
                for kc in range(KC):
                    nc.tensor.matmul(
                        ps[0:TL], xt_sb[kc][:, xo:xo + TL],
                        wv_sb[kc][:, half * VH:(half + 1) * VH],
                        start=(kc == 0), stop=(kc == KC - 1))
                nc.vector.tensor_add(
                    v[0:TL, half * VH:(half + 1) * VH], ps[0:TL],
                    vb_bcast[0:TL, half * VH:(half + 1) * VH])
            v_sbs[b][tt] = v

        def emit_V(xt_sb, par):
            v_sbs = [[None, None], [None, None]]
            for b in range(2):
                for tt in range(2):
                    emit_V_one(xt_sb, par, b, tt, v_sbs)
            return v_sbs

        rs_slot = [0]

        def head_unit(h, qk_sb, v_sbs, ao_sb):
            qq = qk_sb[h // 2]
            kk = qk_sb[KC + h // 2]
            po = (h % 2) * 64
            st = []
            for kt in range(2):
                KT = KT_SZ[kt]
                s = psS.tile([128, F2], f32, tag=f"st{kt}", name=f"st{kt}")
                for b in range(2):
                    nc.tensor.matmul(
                        s[0:KT, b * NN:(b + 1) * NN],
                        kk[po:po + 64, b * NN + kt * 128:b * NN + kt * 128 + KT],
                        qq[po:po + 64, b * NN:(b + 1) * NN],
                        start=(b == 0), stop=(b == 1))
                st.append(s)
            e = ep.tile([128, 2 * F2], f16, tag="e", name="e")
            for kt in range(2):
                nc.scalar.activation(e[:, kt * F2:(kt + 1) * F2], st[kt], Exp)
            ptt = ptp.tile([128, 2 * F2], f16, tag="pt", name="pt")
            nc.vector.tensor_mul(ptt, e, erpb_sb[h])
            pt = [ptt[:, 0:F2], ptt[:, F2:2 * F2]]
            ot = psO.tile([65, F2], f32, tag="ot", name="ot")
            first = True
            for b in range(2):
                for kt in range(2):
                    KT = KT_SZ[kt]
                    nc.tensor.matmul(
                        ot[:, b * NN:(b + 1) * NN],
                        v_sbs[b][kt][0:KT, h * 65:(h + 1) * 65],
                        pt[kt][0:KT, b * NN:(b + 1) * NN],
                        start=first, stop=(b == 1 and kt == 1))
                    first = False
            if NORM_MODE == "off":
                nc.scalar.copy(ao_sb[h // 2][po:po + 64, :], ot[0:64, :])
                return
            if NORM_MODE == "pemm":
                r16 = lrp.tile([1, F2], f16, tag="r16", name="r16")
                nc.vector.reciprocal(r16, ot[64:65, :])
                r_ps = psR.tile([64, F2], f32, tag="rps", name="rps")
                nc.tensor.matmul(r_ps, ones_sb[:, 0:64], r16,
                                 start=True, stop=True)
                nc.vector.tensor_mul(ao_sb[h // 2][po:po + 64, :],
                                     ot[0:64, :], r_ps)
                return
            r_sb = lrp.tile([1, F2], f32, tag="r", name="r")
            nc.vector.reciprocal(r_sb, ot[64:65, :])
            if NORM_MODE == "bcastap":
                rs = r_sb[0:1, :]
                nc.vector.tensor_mul(
                    ao_sb[h // 2][po:po + 64, :], ot[0:64, :],
                    bass.AP(tensor=rs.tensor, offset=rs.offset,
                            ap=[[0, 64]] + list(rs.ap[1:])))
                return
            r_full = rfp.tile([64, F2], f32, tag="rf", name="rf")
            if NORM_MODE == "dvecopy":
                nc.vector.tensor_copy(r_full, vb_bcast[0:64, 0:F2])
            elif NORM_MODE == "dram":
                slot = rs_slot[0] % 16
                rs_slot[0] += 1
                nc.sync.dma_start(out=rs_d[slot], in_=r_sb)
                rd = rs_d[slot][0:1, :]
                nc.sync.dma_start(out=r_full, in_=bass.AP(
                    tensor=rd.tensor, offset=rd.offset,
                    ap=[[0, 64]] + list(rd.ap[1:])))
            else:
                nc.gpsimd.partition_broadcast(r_full, r_sb)
            nc.vector.tensor_mul(ao_sb[h // 2][po:po + 64, :],
                                 ot[0:64, :], r_full)

        def emit_proj(p, ao_sb):
            for b in range(2):
                for tt in range(2):
                    t0 = tt * 128
                    tl = TT_SZ[tt]
                    y_sb = yp.tile([128, C], mdt, tag="y", name="y")
                    for half in range(2):
                        ps = psA.tile([128, PH], f32, tag="mm", name="mm")
                        nc.tensor.matmul(
                            ps[0:tl], ones_sb[:, 0:tl],
                            pb_sb[:, half * PH:(half + 1) * PH],
                            start=True, stop=False)
                        for dc in range(KC):
                            nc.tensor.matmul(
                                ps[0:tl], ao_sb[dc][:, b * NN + t0:b * NN + t0 + tl],
                                pw_sb[dc][:, half * PH:(half + 1) * PH],
                                start=False, stop=(dc == KC - 1))
                        nc.scalar.copy(
                            y_sb[0:tl, half * PH:(half + 1) * PH], ps[0:tl])
                    nc.sync.dma_start(
                        out=y_d[2 * p + b, t0:t0 + tl, :], in_=y_sb[0:tl])

        def whole_pass():
            qk_cur, xt_cur, thunks = build_A(0)
            for t in thunks:
                t()
            v_cur = emit_V(xt_cur, 0)
            for p in range(PAIRS):
                par = p % 2
                if p + 1 < PAIRS:
                    qk_nxt, xt_nxt, a_thunks = build_A(p + 1)
                    v_nxt = [[None, None], [None, None]]

                    def v_thunk(b, tt, p1=p + 1, vn=v_nxt, xs=xt_nxt):
                        emit_V_one(xs, p1 % 2, b, tt, vn)

                    a_thunks = a_thunks + [
                        lambda b=b, tt=tt: v_thunk(b, tt)
                        for b in range(2) for tt in range(2)]
                else:
                    qk_nxt = xt_nxt = v_nxt = None
                    a_thunks = []
                ao_sb = [aop.tile([128, F2], mdt, tag=f"ao{par}_{dc}",
                                  name=f"ao{par}_{dc}") for dc in range(KC)]
                if SKIP_MODE == "dense":
                    # timing probe: skip attention, fill ao with junk
                    for dc in range(KC):
                        nc.vector.tensor_copy(ao_sb[dc], vb_bcast[:, 0:F2])
                    for t in a_thunks:
                        t()
                else:
                    emitted = 0
                    for h in range(H):
                        head_unit(h, qk_cur, v_cur, ao_sb)
                        want = (h + 1) * len(a_thunks) // H
                        while emitted < want:
                            a_thunks[emitted]()
                            emitted += 1
                emit_proj(p, ao_sb)
                qk_cur, xt_cur, v_cur = qk_nxt, xt_nxt, v_nxt

        if PERF_REPS > 0:
            with tc.For_i(0, PERF_REPS, 1):
                whole_pass()
        else:
            for _ in range(int(os.environ.get("PERF_UNROLL", "1"))):
                whole_pass()

    nc.compile()
    _prog_cache[key] = nc
    return nc


def _host_prep(x, qkv_w, q_bias, v_bias, q_lora_a, q_lora_b, k_lora_a,
               k_lora_b, v_lora_a, v_lora_b, rel_pos_table, proj_w, proj_b,
               rel_pos_index):
    import ml_dtypes
    bf16 = ml_dtypes.bfloat16
    f = np.float32
    x = np.asarray(x, f)
    q_bias = np.asarray(q_bias, f)
    proj_w = np.asarray(proj_w, f)
    rel_pos_table = np.asarray(rel_pos_table, f)
    rel_pos_index = np.asarray(rel_pos_index)

    # fold LoRA (x @ A.T @ B.T == x @ (B@A).T) and attention scale into weights
    lora = np.vstack([
        np.asarray(q_lora_b, np.float64) @ np.asarray(q_lora_a, np.float64),
        np.asarray(k_lora_b, np.float64) @ np.asarray(k_lora_a, np.float64),
        np.asarray(v_lora_b, np.float64) @ np.asarray(v_lora_a, np.float64),
    ])
    W = (np.asarray(qkv_w, np.float64) + lora)
    W[0:C] *= SCALE
    W = W.astype(f)

    wqk = np.ascontiguousarray(W[0:2 * C].T)                     # [768, 1536]
    bqk = np.ascontiguousarray(
        np.concatenate([q_bias * SCALE, np.zeros(C, f)]).reshape(NCO, 128).T)

    WvT = W[2 * C:3 * C].T                                       # [768, 768]
    wv = np.zeros((C, VW), f)
    vb = np.zeros((1, VW), f)
    for h in range(H):
        wv[:, h * 65:h * 65 + 64] = WvT[:, h * 64:(h + 1) * 64]
        vb[0, h * 65 + 64] = 1.0
    pw = np.ascontiguousarray(proj_w.T)
    # softmax weights sum to 1 -> v_bias adds a constant to attn_out;
    # fold it into the projection bias: pb = proj_b + proj_w @ v_bias
    pb = (np.asarray(proj_b, f) + proj_w @ np.asarray(v_bias, f)).reshape(1, C)

    # exp(rpb): [h, kt, k_in_chunk, q] duplicated for the two packed batches
    rpb = rel_pos_table[rel_pos_index.reshape(-1).astype(np.int64)]
    rpb = rpb.reshape(NT, NT, H)                                  # [q, k, h]
    erpb_t = np.exp(rpb).transpose(2, 1, 0).astype(f)             # [h, k, q]
    erpb = np.ones((H, 2, 128, NN), f)
    erpb[:, 0, 0:128, :] = erpb_t[:, 0:128, :]
    erpb[:, 1, 0:NT - 128, :] = erpb_t[:, 128:NT, :]
    erpb = np.concatenate([erpb, erpb], axis=3)                   # [h,kt,128,394]
    erpb = np.ascontiguousarray(
        np.concatenate([erpb[:, 0], erpb[:, 1]], axis=2))         # [h,128,788]

    # pack batch pairs side by side along tokens: [B//2, C, 394]
    xt = np.ascontiguousarray(
        x.reshape(B // 2, 2, NN, C).transpose(0, 3, 1, 2).reshape(B // 2, C, F2))

    return {
        "xt": xt.astype(bf16),
        "wqk": wqk.astype(bf16),
        "bqk": bqk,
        "wv": wv.astype(bf16),
        "vb": vb,
        "pw": pw.astype(bf16),
        "pb": pb.astype(np.float16),
        "ones": np.ones((1, 128), np.float16),
        "erpb": erpb.astype(np.float16),
    }


def kernel(**inputs):
    arrs = _host_prep(**inputs)
    nc = _build_program()
    in_maps = []
    ppc = BPC // 2
    for ci in range(NCORES):
        m = dict(arrs)
        m["xt"] = np.ascontiguousarray(arrs["xt"][ci * ppc:(ci + 1) * ppc])
        in_maps.append(m)
    last_exc = None
    for attempt in range(3):
        try:
            res = bass_utils.run_bass_kernel_spmd(
                nc, in_maps, core_ids=list(range(NCORES)))
            break
        except Exception as e:  # transient NRT device flakes recover on retry
            last_exc = e
            import time
            time.sleep(5.0 * (attempt + 1))
    else:
        raise last_exc
    out = np.concatenate([r["y"] for r in res.results], axis=0)
    return out.astype(np.float32)


# revision 12
# speedup vs baseline: 1.2627x; 1.0277x over previous
"""Trainium2 Bass kernel for nn_Attention_LoRA (Swin attention w/ LoRA + rel-pos bias).

v2 design (vs v1: fp32r + 256-pad + per-batch head units):
  - bf16 matmul inputs, fp16 softmax intermediates (DVE 2x mode), f32 psums.
  - No token padding: the two batches of a pair are PACKED along the free
    axis (197+197=394), so every matmul/elementwise op streams only real
    tokens. fp32r's free>=256 restriction is gone with 16-bit inputs.
  - One head unit covers BOTH batches of the pair: half the units, half the
    recip/broadcast/normalize ops of v1.
  - v-bias (the softmax-denominator ones columns) applied during the DVE
    PSUM->SBUF evacuation instead of an extra PE matmul; proj bias applied
    by a rank-1 fp16 matmul folded into the proj accumulation.
  Per batch: qkT[c,t] = Wqk @ xT; v[t,c'] = x @ WvT (65-wide head blocks,
  ones col -> PV row 64 = softmax denom l); ST[k,q] per head; PT = exp(ST)
  * exp(rpb); OT = v.T @ PT; ao = OT[0:64]/l; y = ao.T @ projW + pb.
  Engines execute in emission order, so latency chains are software-
  pipelined: heads split into front (S->exp->*erpb) and back (PV->
  normalize) emitted TWO heads apart, with next-pair QK/V matmul groups
  and the PREVIOUS pair's projection interleaved between head units as
  PE filler. Measured on trn2 (8 cores, axon, paired For_i-delta
  R2=8192): ~273-295 us per core for its 8-batch share (v1 fp32r
  baseline: 535-596 us same method); scaled absmax err ~4.4e-3.
"""
import os
from contextlib import ExitStack

import numpy as np

import concourse.bacc as bacc
import concourse.bass as bass
import concourse.mybir as mybir
import concourse.tile as tile
from concourse import bass_utils

B, NT, C, H, WS, RANK = 64, 197, 768, 12, 14, 24
HD = C // H
SCALE = HD ** -0.5
NCORES = 8
BPC = B // NCORES          # batches per core
NN = NT                    # real token count (no padding)
F2 = 2 * NN                # 394: two batches packed along the free axis
KC = C // 128              # 6 contraction chunks
NCO = (2 * C) // 128       # 12 q+k output chunks
VW = H * (HD + 1)          # 780: v head-blocks of 65 (64 v cols + ones col)
VH = VW // 2               # 390
PH = C // 2                # 384
KT_SZ = [128, NT - 128]    # keys chunks: 128, 69
TT_SZ = [128, NT - 128]    # token chunks: 128, 69

PERF_REPS = int(os.environ.get("PERF_REPS", "0"))
NORM_MODE = os.environ.get("NORM_MODE", "gps")  # gps | dram | dvecopy | off
SKIP_MODE = os.environ.get("SKIP_MODE", "full")  # full | dense (timing probe)

_prog_cache = {}


def _build_program():
    key = (PERF_REPS, NORM_MODE, SKIP_MODE)
    if key in _prog_cache:
        return _prog_cache[key]

    f32 = mybir.dt.float32
    f16 = mybir.dt.float16
    mdt = mybir.dt.bfloat16
    Exp = mybir.ActivationFunctionType.Exp

    nc = bacc.Bacc("TRN2", target_bir_lowering=False, debug=False,
                   num_devices=NCORES)
    xt_d = nc.dram_tensor("xt", [BPC // 2, C, F2], mdt,
                          kind="ExternalInput").ap()
    wqk_d = nc.dram_tensor("wqk", [C, 2 * C], mdt, kind="ExternalInput").ap()
    bqk_d = nc.dram_tensor("bqk", [128, NCO], f32, kind="ExternalInput").ap()
    wv_d = nc.dram_tensor("wv", [C, VW], mdt, kind="ExternalInput").ap()
    vb_d = nc.dram_tensor("vb", [1, VW], f32, kind="ExternalInput").ap()
    pw_d = nc.dram_tensor("pw", [C, C], mdt, kind="ExternalInput").ap()
    pb_d = nc.dram_tensor("pb", [1, C], f16, kind="ExternalInput").ap()
    ones_d = nc.dram_tensor("ones", [1, 128], f16, kind="ExternalInput").ap()
    erpb_d = nc.dram_tensor("erpb", [H, 128, 2 * F2], f16,
                            kind="ExternalInput").ap()
    rs_d = nc.dram_tensor("rs", [16, 1, F2], f32, kind="Internal").ap()
    y_d = nc.dram_tensor("y", [BPC, NT, C], mdt, kind="ExternalOutput").ap()

    with tile.TileContext(nc) as tc, ExitStack() as ctx, \
            nc.allow_low_precision("bf16/fp16 matmul + softmax by design"):
        consts = ctx.enter_context(tc.tile_pool(name="consts", bufs=1))
        xp = ctx.enter_context(tc.tile_pool(name="xp", bufs=1))
        qkp = ctx.enter_context(tc.tile_pool(name="qkp", bufs=1))
        vp = ctx.enter_context(tc.tile_pool(name="vp", bufs=1))
        ep = ctx.enter_context(tc.tile_pool(name="ep", bufs=3))
        ptp = ctx.enter_context(tc.tile_pool(name="ptp", bufs=3))
        lrp = ctx.enter_context(tc.tile_pool(name="lrp", bufs=3))
        rfp = ctx.enter_context(tc.tile_pool(name="rfp", bufs=3))
        aop = ctx.enter_context(tc.tile_pool(name="aop", bufs=1))
        yp = ctx.enter_context(tc.tile_pool(name="yp", bufs=2))
        psA = ctx.enter_context(tc.tile_pool(name="psA", bufs=2, space="PSUM"))
        psS = ctx.enter_context(tc.tile_pool(
            name="psS", bufs=(1 if NORM_MODE == "pemm" else 2), space="PSUM"))
        psO = ctx.enter_context(tc.tile_pool(name="psO", bufs=2, space="PSUM"))
        psR = (ctx.enter_context(tc.tile_pool(name="psR", bufs=2, space="PSUM"))
               if NORM_MODE == "pemm" else None)

        # ---- constants ----
        wqk_sb = []
        wv_sb = []
        pw_sb = []
        for kc in range(KC):
            t = consts.tile([128, 2 * C], mdt, tag=f"wqk{kc}", name=f"wqk{kc}")
            nc.sync.dma_start(out=t, in_=wqk_d[kc * 128:(kc + 1) * 128, :])
            wqk_sb.append(t)
        for kc in range(KC):
            t = consts.tile([128, VW], mdt, tag=f"wv{kc}", name=f"wv{kc}")
            nc.sync.dma_start(out=t, in_=wv_d[kc * 128:(kc + 1) * 128, :])
            wv_sb.append(t)
        for kc in range(KC):
            t = consts.tile([128, C], mdt, tag=f"pw{kc}", name=f"pw{kc}")
            nc.sync.dma_start(out=t, in_=pw_d[kc * 128:(kc + 1) * 128, :])
            pw_sb.append(t)
        vb_bcast = consts.tile([128, VW], f32, tag="vbb", name="vbb")
        nc.sync.dma_start(out=vb_bcast, in_=bass.AP(
            tensor=vb_d.tensor, offset=vb_d.offset,
            ap=[[0, 128]] + list(vb_d.ap[1:])))
        pb_sb = consts.tile([1, C], f16, tag="pb", name="pb")
        nc.sync.dma_start(out=pb_sb, in_=pb_d)
        ones_sb = consts.tile([1, 128], f16, tag="ones", name="ones")
        nc.sync.dma_start(out=ones_sb, in_=ones_d)
        bqk_sb = consts.tile([128, NCO], f32, tag="bqk", name="bqk")
        nc.sync.dma_start(out=bqk_sb, in_=bqk_d)
        erpb_sb = {}
        for h in range(H):
            t = consts.tile([128, 2 * F2], f16, tag=f"erpb{h}",
                            name=f"erpb{h}")
            nc.sync.dma_start(out=t, in_=erpb_d[h, :, :])
            erpb_sb[h] = t

        PAIRS = BPC // 2
        co_order = [c for pr in zip(range(KC), range(KC, NCO)) for c in pr]

        def build_A(p):
            """Emit pair p's xt DMAs now; return (qk_sb, xt_sb, thunks)."""
            par = p % 2
            xt_sb = []
            for kc in range(KC):
                t = xp.tile([128, F2], mdt, tag=f"x{par}_{kc}",
                            name=f"x{par}_{kc}")
                nc.sync.dma_start(out=t, in_=xt_d[p, kc * 128:(kc + 1) * 128, :])
                xt_sb.append(t)
            qk_sb = [None] * NCO

            def qk_thunk(co):
                ps = psA.tile([128, F2], f32, tag="mm", name="mm")
                for kc in range(KC):
                    nc.tensor.matmul(
                        ps, wqk_sb[kc][:, co * 128:(co + 1) * 128], xt_sb[kc],
                        start=(kc == 0), stop=(kc == KC - 1))
                qk = qkp.tile([128, F2], mdt, tag=f"qk{par}_{co}",
                              name=f"qk{par}_{co}")
                nc.scalar.add(qk, ps, add=bqk_sb[:, co:co + 1])
                qk_sb[co] = qk

            thunks = [lambda co=co: qk_thunk(co) for co in co_order]
            return qk_sb, xt_sb, thunks

        def emit_V_one(xt_sb, par, b, tt, v_sbs):
            """v[b][tt] = [tokens<=128, 780] bf16 (ones cols via vb add)."""
            TL = TT_SZ[tt]
            xo = b * NN + tt * 128
            v = vp.tile([128, VW], mdt, tag=f"v{par}_{b}_{tt}",
                        name=f"v{par}_{b}_{tt}")
            for half in range(2):
                ps = psA.tile([128, VH], f32, tag="mm", name="mm")
                for kc in range(KC):
                    nc.tensor.matmul(
                        ps[0:TL], xt_sb[kc][:, xo:xo + TL],
                        wv_sb[kc][:, half * VH:(half + 1) * VH],
                        start=(kc == 0), stop=(kc == KC - 1))
                nc.vector.tensor_add(
                    v[0:TL, half * VH:(half + 1) * VH], ps[0:TL],
                    vb_bcast[0:TL, half * VH:(half + 1) * VH])
            v_sbs[b][tt] = v

        def emit_V(xt_sb, par):
            v_sbs = [[None, None], [None, None]]
            for b in range(2):
                for tt in range(2):
                    emit_V_one(xt_sb, par, b, tt, v_sbs)
            return v_sbs

        rs_slot = [0]
        pend = {}

        def head_front(h, qk_sb):
            qq = qk_sb[h // 2]
            kk = qk_sb[KC + h // 2]
            po = (h % 2) * 64
            st = []
            for kt in range(2):
                KT = KT_SZ[kt]
                s = psS.tile([128, F2], f32, tag=f"st{kt}", name=f"st{kt}")
                for b in range(2):
                    nc.tensor.matmul(
                        s[0:KT, b * NN:(b + 1) * NN],
                        kk[po:po + 64, b * NN + kt * 128:b * NN + kt * 128 + KT],
                        qq[po:po + 64, b * NN:(b + 1) * NN],
                        start=(b == 0), stop=(b == 1))
                st.append(s)
            e = ep.tile([128, 2 * F2], f16, tag="e", name="e")
            for kt in range(2):
                nc.scalar.activation(e[:, kt * F2:(kt + 1) * F2], st[kt], Exp)
            ptt = ptp.tile([128, 2 * F2], f16, tag="pt", name="pt")
            nc.vector.tensor_mul(ptt, e, erpb_sb[h])
            pend[h] = ptt

        def head_back(h, v_sbs, ao_sb):
            po = (h % 2) * 64
            ptt = pend.pop(h)
            pt = [ptt[:, 0:F2], ptt[:, F2:2 * F2]]
            ot = psO.tile([65, F2], f32, tag="ot", name="ot")
            first = True
            for b in range(2):
                for kt in range(2):
                    KT = KT_SZ[kt]
                    nc.tensor.matmul(
                        ot[:, b * NN:(b + 1) * NN],
                        v_sbs[b][kt][0:KT, h * 65:(h + 1) * 65],
                        pt[kt][0:KT, b * NN:(b + 1) * NN],
                        start=first, stop=(b == 1 and kt == 1))
                    first = False
            if NORM_MODE == "off":
                nc.scalar.copy(ao_sb[h // 2][po:po + 64, :], ot[0:64, :])
                return
            if NORM_MODE == "pemm":
                r16 = lrp.tile([1, F2], f16, tag="r16", name="r16")
                nc.vector.reciprocal(r16, ot[64:65, :])
                r_ps = psR.tile([64, F2], f32, tag="rps", name="rps")
                nc.tensor.matmul(r_ps, ones_sb[:, 0:64], r16,
                                 start=True, stop=True)
                nc.vector.tensor_mul(ao_sb[h // 2][po:po + 64, :],
                                     ot[0:64, :], r_ps)
                return
            r_sb = lrp.tile([1, F2], f32, tag="r", name="r")
            nc.vector.reciprocal(r_sb, ot[64:65, :])
            if NORM_MODE == "bcastap":
                rs = r_sb[0:1, :]
                nc.vector.tensor_mul(
                    ao_sb[h // 2][po:po + 64, :], ot[0:64, :],
                    bass.AP(tensor=rs.tensor, offset=rs.offset,
                            ap=[[0, 64]] + list(rs.ap[1:])))
                return
            r_full = rfp.tile([64, F2], f32, tag="rf", name="rf")
            if NORM_MODE == "dvecopy":
                nc.vector.tensor_copy(r_full, vb_bcast[0:64, 0:F2])
            elif NORM_MODE == "dram":
                slot = rs_slot[0] % 16
                rs_slot[0] += 1
                nc.sync.dma_start(out=rs_d[slot], in_=r_sb)
                rd = rs_d[slot][0:1, :]
                nc.sync.dma_start(out=r_full, in_=bass.AP(
                    tensor=rd.tensor, offset=rd.offset,
                    ap=[[0, 64]] + list(rd.ap[1:])))
            else:
                nc.gpsimd.partition_broadcast(r_full, r_sb)
            nc.vector.tensor_mul(ao_sb[h // 2][po:po + 64, :],
                                 ot[0:64, :], r_full)

        def proj_one(p, ao_sb, b, tt):
            t0 = tt * 128
            tl = TT_SZ[tt]
            y_sb = yp.tile([128, C], mdt, tag="y", name="y")
            for half in range(2):
                ps = psA.tile([128, PH], f32, tag="mm", name="mm")
                nc.tensor.matmul(
                    ps[0:tl], ones_sb[:, 0:tl],
                    pb_sb[:, half * PH:(half + 1) * PH],
                    start=True, stop=False)
                for dc in range(KC):
                    nc.tensor.matmul(
                        ps[0:tl], ao_sb[dc][:, b * NN + t0:b * NN + t0 + tl],
                        pw_sb[dc][:, half * PH:(half + 1) * PH],
                        start=False, stop=(dc == KC - 1))
                nc.scalar.copy(
                    y_sb[0:tl, half * PH:(half + 1) * PH], ps[0:tl])
            nc.sync.dma_start(
                out=y_d[2 * p + b, t0:t0 + tl, :], in_=y_sb[0:tl])

        def proj_thunks(p, ao_sb):
            return [lambda b=b, tt=tt: proj_one(p, ao_sb, b, tt)
                    for b in range(2) for tt in range(2)]

        def emit_proj(p, ao_sb):
            for t in proj_thunks(p, ao_sb):
                t()

        def whole_pass():
            qk_cur, xt_cur, thunks = build_A(0)
            for t in thunks:
                t()
            v_cur = emit_V(xt_cur, 0)
            proj_prev = []
            for p in range(PAIRS):
                par = p % 2
                if p + 1 < PAIRS:
                    qk_nxt, xt_nxt, a_thunks = build_A(p + 1)
                    v_nxt = [[None, None], [None, None]]

                    def v_thunk(b, tt, p1=p + 1, vn=v_nxt, xs=xt_nxt):
                        emit_V_one(xs, p1 % 2, b, tt, vn)

                    a_thunks = a_thunks + [
                        lambda b=b, tt=tt: v_thunk(b, tt)
                        for b in range(2) for tt in range(2)]
                else:
                    qk_nxt = xt_nxt = v_nxt = None
                    a_thunks = []
                ao_sb = [aop.tile([128, F2], mdt, tag=f"ao{par}_{dc}",
                                  name=f"ao{par}_{dc}") for dc in range(KC)]
                if SKIP_MODE == "dense":
                    # timing probe: skip attention, fill ao with junk
                    for dc in range(KC):
                        nc.vector.tensor_copy(ao_sb[dc], vb_bcast[:, 0:F2])
                    for t in a_thunks:
                        t()
                else:
                    # interleave: next pair's QK/V thunks + prev pair's proj
                    fill = []
                    na, npj = len(a_thunks), len(proj_prev)
                    for i in range(max(na, npj)):
                        if i < na:
                            fill.append(a_thunks[i])
                        if i < npj:
                            fill.append(proj_prev[i])
                    emitted = 0
                    for j in range(H + 2):
                        if j < H:
                            head_front(j, qk_cur)
                        if j >= 2:
                            head_back(j - 2, v_cur, ao_sb)
                        want = (j + 1) * len(fill) // (H + 2)
                        while emitted < want:
                            fill[emitted]()
                            emitted += 1
                proj_prev = proj_thunks(p, ao_sb)
                qk_cur, xt_cur, v_cur = qk_nxt, xt_nxt, v_nxt
            for t in proj_prev:
                t()

        if PERF_REPS > 0:
            with tc.For_i(0, PERF_REPS, 1):
                whole_pass()
        else:
            for _ in range(int(os.environ.get("PERF_UNROLL", "1"))):
                whole_pass()

    nc.compile()
    _prog_cache[key] = nc
    return nc


def _host_prep(x, qkv_w, q_bias, v_bias, q_lora_a, q_lora_b, k_lora_a,
               k_lora_b, v_lora_a, v_lora_b, rel_pos_table, proj_w, proj_b,
               rel_pos_index):
    import ml_dtypes
    bf16 = ml_dtypes.bfloat16
    f = np.float32
    x = np.asarray(x, f)
    q_bias = np.asarray(q_bias, f)
    proj_w = np.asarray(proj_w, f)
    rel_pos_table = np.asarray(rel_pos_table, f)
    rel_pos_index = np.asarray(rel_pos_index)

    # fold LoRA (x @ A.T @ B.T == x @ (B@A).T) and attention scale into weights
    lora = np.vstack([
        np.asarray(q_lora_b, np.float64) @ np.asarray(q_lora_a, np.float64),
        np.asarray(k_lora_b, np.float64) @ np.asarray(k_lora_a, np.float64),
        np.asarray(v_lora_b, np.float64) @ np.asarray(v_lora_a, np.float64),
    ])
    W = (np.asarray(qkv_w, np.float64) + lora)
    W[0:C] *= SCALE
    W = W.astype(f)

    wqk = np.ascontiguousarray(W[0:2 * C].T)                     # [768, 1536]
    bqk = np.ascontiguousarray(
        np.concatenate([q_bias * SCALE, np.zeros(C, f)]).reshape(NCO, 128).T)

    WvT = W[2 * C:3 * C].T                                       # [768, 768]
    wv = np.zeros((C, VW), f)
    vb = np.zeros((1, VW), f)
    for h in range(H):
        wv[:, h * 65:h * 65 + 64] = WvT[:, h * 64:(h + 1) * 64]
        vb[0, h * 65 + 64] = 1.0
    pw = np.ascontiguousarray(proj_w.T)
    # softmax weights sum to 1 -> v_bias adds a constant to attn_out;
    # fold it into the projection bias: pb = proj_b + proj_w @ v_bias
    pb = (np.asarray(proj_b, f) + proj_w @ np.asarray(v_bias, f)).reshape(1, C)

    # exp(rpb): [h, kt, k_in_chunk, q] duplicated for the two packed batches
    rpb = rel_pos_table[rel_pos_index.reshape(-1).astype(np.int64)]
    rpb = rpb.reshape(NT, NT, H)                                  # [q, k, h]
    erpb_t = np.exp(rpb).transpose(2, 1, 0).astype(f)             # [h, k, q]
    erpb = np.ones((H, 2, 128, NN), f)
    erpb[:, 0, 0:128, :] = erpb_t[:, 0:128, :]
    erpb[:, 1, 0:NT - 128, :] = erpb_t[:, 128:NT, :]
    erpb = np.concatenate([erpb, erpb], axis=3)                   # [h,kt,128,394]
    erpb = np.ascontiguousarray(
        np.concatenate([erpb[:, 0], erpb[:, 1]], axis=2))         # [h,128,788]

    # pack batch pairs side by side along tokens: [B//2, C, 394]
    xt = np.ascontiguousarray(
        x.reshape(B // 2, 2, NN, C).transpose(0, 3, 1, 2).reshape(B // 2, C, F2))

    return {
        "xt": xt.astype(bf16),
        "wqk": wqk.astype(bf16),
        "bqk": bqk,
        "wv": wv.astype(bf16),
        "vb": vb,
        "pw": pw.astype(bf16),
        "pb": pb.astype(np.float16),
        "ones": np.ones((1, 128), np.float16),
        "erpb": erpb.astype(np.float16),
    }


def kernel(**inputs):
    arrs = _host_prep(**inputs)
    nc = _build_program()
    in_maps = []
    ppc = BPC // 2
    for ci in range(NCORES):
        m = dict(arrs)
        m["xt"] = np.ascontiguousarray(arrs["xt"][ci * ppc:(ci + 1) * ppc])
        in_maps.append(m)
    last_exc = None
    for attempt in range(3):
        try:
            res = bass_utils.run_bass_kernel_spmd(
                nc, in_maps, core_ids=list(range(NCORES)))
            break
        except Exception as e:  # transient NRT device flakes recover on retry
            last_exc = e
            import time
            time.sleep(5.0 * (attempt + 1))
    else:
        raise last_exc
    out = np.concatenate([r["y"] for r in res.results], axis=0)
    return out.astype(np.float32)
